# revision 10
# baseline (speedup 1.0000x reference)
"""CURL color-pipeline kernel for 8 TRN2 NeuronCores (Bass/Tile).

Contract: kernel(img, L, R, S) takes the FULL inputs (img (3,2048,2048) f32,
L/R/S curve params), returns (out_image (3,2048,2048) f32, slope_sqr_diff f32
scalar) exactly like the reference.

Strategy:
  - The regulariser ssd and all per-curve scalars (A_c, B_c with
    scale = A + B*x) depend only on the 160 curve parameters -> computed on
    host in float32 with reference-faithful op order, baked into the device
    program as immediates.
  - Image work is purely per-pixel. Shard H across the 8 cores
    (each core: (3, 256, 2048) = 3 planes of [128 partitions x 4096]).
  - Per core, process 4 column-chunks of [128, 1024] through a fused
    DVE/ACT/GPSIMD schedule (no PE: fp32r matmul rounds inputs to 11
    mantissa bits, too coarse).
"""
import numpy as np

import concourse.bacc as bacc
import concourse.bass as bass
import concourse.mybir as mybir
import concourse.tile as tile
from concourse.bass_utils import run_bass_kernel_spmd

F32 = mybir.dt.float32
U8 = mybir.dt.uint8
I32_ = mybir.dt.int32
OP = mybir.AluOpType
AF = mybir.ActivationFunctionType

P = 128          # partitions
FREE = 4096      # free dim per plane per core
CHUNK = 2048     # columns per compute chunk
NCHUNK = FREE // CHUNK
NCORE = 8
H, W = 2048, 2048
HSH = H // NCORE  # rows per core

EPS = np.float32(6.0 / 29.0)
WPS = [np.float32(0.950456), np.float32(1.0), np.float32(1.088754)]
M2 = np.float32([[3.2404542, -0.969266, 0.0556434],
                 [-1.5371385, 1.8760108, -0.2040259],
                 [-0.4985314, 0.041556, 1.0572252]])
GAM_C = np.float32(0.0031308)
GAM_BITS = int(GAM_C.view(np.int32))
I32 = None  # set below

USE_GPSIMD = True


def _curve_coeffs(Pv, n):
    """Per-curve (A, B) with scale = A + B*x, and the ssd term; float32
    semantics faithful to the reference."""
    out = []
    ssd = np.float32(0.0)
    ncurves = Pv.shape[0] // n
    for c in range(ncurves):
        C = np.exp(Pv[c * n:(c + 1) * n].astype(np.float32)).astype(np.float32)
        K = C.shape[0]
        slope = (C[1:] - C[:-1]).astype(np.float32)
        d = np.float32(np.sum(((slope[1:] - slope[:-1]).astype(np.float32) ** 2).astype(np.float32),
                              dtype=np.float32))
        ssd = np.float32(ssd + d)
        s = slope[:-1]
        B = np.float32(np.float32(K - 1) * np.float32(np.sum(s, dtype=np.float32)))
        A = np.float32(C[0] - np.float32(np.dot(s, np.arange(K - 2, dtype=np.float32))))
        out.append((float(A), float(B)))
    return out, ssd


def build_program(labc, rgbc, hsvc):
    """Build the single-core Bass program (same NEFF runs SPMD on all 8).

    SBUF slot plan (all [P, CHUNK] fp32, pool tags, bufs=1 unless noted):
      io: x0 x1 x2 (bufs=2)          raw input channels, die after S1
      Y0 Y1 Y2: y (S1) -> gamma t/p/pw (S2d) -> mx/mn/df (S4) -> ra/v1/s2 ...
      F0 F1 F2: f (S2a) -> rgb (S2c) -> inv/sv/hb (S4) -> u4/u5/cc (S6)
      A0 A1 A2: a_i (S2b) -> gq (S2d) -> hg/hr (S4) -> vs/ga/h1->ba (S5/6)
      B0 B1 B2: b0/xyz (S2b/c) -> r,g,b (S3) -> u1/u2/u3 (S6)
    Exact lifetime order is load-bearing; validated against golden in sim.
    """
    nc = bacc.Bacc("TRN2", target_bir_lowering=False, debug=False)

    img_d = nc.dram_tensor("img", (3, P, FREE), F32, kind="ExternalInput").ap()
    out_d = nc.dram_tensor("out", (3, P, FREE), F32, kind="ExternalOutput").ap()

    with tile.TileContext(nc) as tc:
        import contextlib
        ctx = contextlib.ExitStack()
        with ctx:
            io = ctx.enter_context(tc.tile_pool(name="io", bufs=2))
            wk = ctx.enter_context(tc.tile_pool(name="wk", bufs=1))
            mk = ctx.enter_context(tc.tile_pool(name="mk", bufs=2))
            ac = ctx.enter_context(tc.tile_pool(name="ac", bufs=2))

            ve = nc.vector
            se = nc.scalar
            ge = nc.gpsimd if USE_GPSIMD else nc.vector

            def amr(out, in0, in1, scale, bias):
                a = ac.tile([P, 1], F32, tag="acc", name="accd")
                ve.affine_mul_reduce(out, a[:], in0, in1, scale, bias)

            # Relu bias constants -k for the hsv->rgb windows
            relu_bias = []
            for k in range(1, 6):
                t = ac.tile([P, 1], F32, tag=f"rb{k}", bufs=1, name=f"rb{k}_")
                nc.gpsimd.memset(t[:], float(-k))
                relu_bias.append(t)

            for ci in range(NCHUNK):
                sl = bass.ts(ci, CHUNK)

                def slot(tag, bufs=None, name=None):
                    return wk.tile([P, CHUNK], F32, tag=tag, bufs=bufs,
                                   name=name or (tag + "_"))

                x = [io.tile([P, CHUNK], F32, tag=f"x{c}", name=f"x{c}_") for c in range(3)]
                for c in range(3):
                    nc.sync.dma_start(x[c][:], img_d[c][:, sl])

                # ---- S1: LAB curves  y = clip((x*B+A)*x, 0, 1)   [slots Y]
                y = [slot(f"Y{c}") for c in range(3)]
                for c, (A, B) in enumerate(labc):
                    amr(y[c][:], x[c][:], x[c][:], B, A)
                    ge.tensor_scalar(y[c][:], y[c][:], 0.0, 1.0, OP.max, OP.min)

                # ---- S2a: f's [slots F]; fy=F1 fx=F0 fz=F2
                f = [slot(f"F{i}") for i in range(3)]
                se.activation(f[1][:], y[0][:], AF.Copy, bias=16.0 / 116.0, scale=100.0 / 116.0)
                ve.affine_then_add(f[0][:], y[1][:], f[1][:], 220.0 / 500.0, -110.0 / 500.0)
                ve.affine_then_add(f[2][:], y[2][:], f[1][:], -220.0 / 200.0, 110.0 / 200.0)

                # ---- S2b: a_i in [slots A] (rw->ln->exp in place);
                #           b0->xyz in [slots B]
                av = [slot(f"A{i}") for i in range(3)]
                xyz = [slot(f"B{i}") for i in range(3)]
                for i in range(3):
                    w3 = float(np.float32(WPS[i] ** (1.0 / 3.0)))
                    ve.tensor_scalar(av[i][:], f[i][:], float(EPS), w3, OP.max, OP.mult)
                    se.activation(av[i][:], av[i][:], AF.Ln)
                    se.activation(av[i][:], av[i][:], AF.Exp, scale=3.0)
                    ve.tensor_scalar(xyz[i][:], f[i][:], float(-EPS), 0.0, OP.add, OP.min)
                    gamma_i = float(np.float32(3.0) * EPS * EPS * WPS[i])
                    ve.affine_then_add(xyz[i][:], xyz[i][:], av[i][:], gamma_i, 0.0)

                # ---- S2c: mix -> rgb_j reuses [slots F] (f dead)
                rgb = [slot(f"F{j}", name=f"rgb{j}_") for j in range(3)]
                for j in range(3):
                    se.activation(rgb[j][:], xyz[0][:], AF.Copy, bias=0.0, scale=float(M2[0, j]))
                    ve.affine_then_add(rgb[j][:], xyz[1][:], rgb[j][:], float(M2[1, j]), 0.0)
                    ve.affine_then_add(rgb[j][:], xyz[2][:], rgb[j][:], float(M2[2, j]), 0.0)

                # ---- S2d: gamma; t/p/pw in [slots Y] (y dead), gq=lin in [slots A] (a dead)
                tt = [slot(f"Y{j}", name=f"t{j}_") for j in range(3)]
                gq = [slot(f"A{j}", name=f"gq{j}_") for j in range(3)]
                for j in range(3):
                    ve.tensor_scalar(tt[j][:], rgb[j][:], 1e-4, None, OP.max)
                    se.activation(tt[j][:], tt[j][:], AF.Ln)
                    se.activation(tt[j][:], tt[j][:], AF.Exp, scale=1.0 / 2.4)
                    se.activation(tt[j][:], tt[j][:], AF.Copy, bias=-0.055, scale=1.055)
                    se.activation(gq[j][:], rgb[j][:], AF.Copy, bias=0.0, scale=12.92)
                    m = mk.tile([P, CHUNK], I32_, tag="gm", name="gm_")
                    ge.tensor_scalar(m[:], rgb[j][:].bitcast(I32_), GAM_BITS, None, OP.is_gt)
                    ve.copy_predicated(gq[j][:], m[:], tt[j][:])

                # ---- S3: RGB curves -> r,g,b in [slots B] (xyz dead)
                (A0, B0), (A1, B1), (A2, B2) = rgbc
                rcv = [slot(f"B{j}", name=f"rc{j}_") for j in range(3)]
                amr(rcv[0][:], gq[0][:], gq[0][:], B0, A0)
                ge.tensor_scalar(gq[1][:], gq[1][:], 0.0, 1.0, OP.max, OP.min)
                amr(rcv[1][:], gq[1][:], gq[1][:], B1, A1)
                ge.tensor_scalar(gq[2][:], gq[2][:], 0.0, 1.0, OP.max, OP.min)
                amr(rcv[2][:], gq[2][:], gq[2][:], B2, A2)
                for j in range(3):
                    ve.tensor_scalar(rcv[j][:], rcv[j][:], 1e-9, 1.0, OP.max, OP.min)
                r, g, b = rcv

                # ---- S4: rgb -> hsv; mx/mn/df in [slots Y] (t dead);
                #          inv/sv/hb in [slots F] (rgb dead); hg/hr in A1/A2 (gq dead
                #          after S3; gq0 dead after its amr)
                mx = slot("Y0", name="mx_")
                mn = slot("Y1", name="mn_")
                ve.tensor_tensor(mx[:], r[:], g[:], OP.max)
                ve.tensor_tensor(mx[:], mx[:], b[:], OP.max)
                ve.tensor_tensor(mn[:], r[:], g[:], OP.min)
                ve.tensor_tensor(mn[:], mn[:], b[:], OP.min)
                df = slot("Y2", name="df_")
                ge.tensor_tensor(df[:], mx[:], mn[:], OP.subtract)
                ge.tensor_scalar(df[:], df[:], 1e-10, None, OP.add)
                inv = slot("F0", name="inv_")
                ve.reciprocal_approx_fast(inv[:], df[:])
                sv = slot("F1", name="sv_")
                ve.reciprocal_approx_fast(sv[:], mx[:])
                ve.tensor_tensor(sv[:], df[:], sv[:], OP.mult)   # s = df/mx

                hb = slot("F2", name="hb_")
                ge.tensor_tensor(hb[:], r[:], g[:], OP.subtract)
                ve.scalar_tensor_tensor(hb[:], hb[:], 1.0 / 6.0, inv[:], OP.mult, OP.mult)
                se.activation(hb[:], hb[:], AF.Copy, bias=2.0 / 3.0, scale=1.0)
                hg = slot("A0", name="hg_")
                ge.tensor_tensor(hg[:], b[:], r[:], OP.subtract)
                ve.scalar_tensor_tensor(hg[:], hg[:], 1.0 / 6.0, inv[:], OP.mult, OP.mult)
                se.activation(hg[:], hg[:], AF.Copy, bias=1.0 / 3.0, scale=1.0)
                hr = slot("A1", name="hr_")
                ge.tensor_tensor(hr[:], g[:], b[:], OP.subtract)
                ve.scalar_tensor_tensor(hr[:], hr[:], 1.0 / 6.0, inv[:], OP.mult, OP.mult)
                ve.scalar_tensor_tensor(hr[:], hr[:], 0.0, hr[:], OP.is_lt, OP.add)
                mg = mk.tile([P, CHUNK], U8, tag="mg", name="mg_")
                ve.tensor_tensor(mg[:], g[:], mx[:], OP.is_equal)
                mr = mk.tile([P, CHUNK], U8, tag="mr", name="mr_")
                ve.tensor_tensor(mr[:], r[:], mx[:], OP.is_equal)
                ve.copy_predicated(hb[:], mg[:], hg[:])
                ve.copy_predicated(hb[:], mr[:], hr[:])
                # h = hb(F2) ; v = mx(Y0) ; s = sv(F1)

                # ---- S5: HSV curves; h1 in A2 (gq2 dead); s1/s2 in Y2 (df dead);
                #          v1 in Y1 (mn dead)
                (Ah, Bh), (As1, Bs1), (As2, Bs2), (Av, Bv) = hsvc
                h1 = slot("A2", name="h1_")
                amr(h1[:], hb[:], hb[:], Bh, Ah)
                ge.tensor_scalar(h1[:], h1[:], 0.0, 1.0, OP.max, OP.min)
                s2t = slot("Y2", name="s2_")
                amr(s2t[:], h1[:], sv[:], Bs1, As1)
                ve.tensor_scalar(s2t[:], s2t[:], 0.0, 1.0, OP.max, OP.min)
                amr(s2t[:], s2t[:], s2t[:], Bs2, As2)
                ve.tensor_scalar(s2t[:], s2t[:], 0.0, 1.0, OP.max, OP.min)
                v1 = slot("Y1", name="v1_")
                amr(v1[:], mx[:], mx[:], Bv, Av)
                ge.tensor_scalar(v1[:], v1[:], 0.0, 1.0, OP.max, OP.min)

                # ---- S6: hsv -> rgb
                # u1..u3 in [slots B] (r,g,b dead), u4 in F0 (inv dead), u5 in F1 (sv dead)
                u1 = slot("B0", name="u1_"); u2 = slot("B1", name="u2_")
                u3 = slot("B2", name="u3_"); u4 = slot("F0", name="u4_")
                u5 = slot("F1", name="u5_")
                for k, ut in enumerate((u1, u2, u3, u4, u5), start=1):
                    se.activation(ut[:], h1[:], AF.Relu, bias=relu_bias[k - 1][:], scale=6.0)
                cc = slot("F2", name="cc_")      # hb dead (bufs=2: crosses to DMA epoch)
                ve.tensor_tensor(cc[:], v1[:], s2t[:], OP.mult)
                vs = slot("A0", name="vs_")              # hg dead
                ve.affine_then_add(vs[:], cc[:], v1[:], -1.0, 0.0)

                ga = slot("A1", name="ga_")      # hr dead
                ve.scalar_tensor_tensor(ga[:], h1[:], 6.0, u1[:], OP.mult, OP.subtract)
                ve.affine_then_add(ga[:], u3[:], ga[:], -1.0, 0.0)
                ve.affine_then_add(ga[:], u4[:], ga[:], 1.0, 0.0)
                ra = slot("Y0", name="ra_")      # mx dead (after v1 amr)
                ge.tensor_tensor(ra[:], u2[:], u1[:], OP.subtract)
                ve.affine_then_add(ra[:], u4[:], ra[:], 1.0, 0.0)
                ve.affine_then_add(ra[:], u5[:], ra[:], -1.0, 0.0)
                ba = slot("A2", name="ba_")      # h1 dead (after u_k + ga STT)
                ge.tensor_tensor(ba[:], u2[:], u3[:], OP.subtract)
                ve.affine_then_add(ba[:], u5[:], ba[:], -1.0, 0.0)

                ve.tensor_tensor(ra[:], cc[:], ra[:], OP.mult)
                ve.tensor_tensor(ra[:], ra[:], v1[:], OP.add)
                ge.tensor_scalar(ra[:], ra[:], 0.0, 1.0, OP.max, OP.min)
                nc.sync.dma_start(out_d[0][:, sl], ra[:])
                ve.tensor_tensor(ga[:], cc[:], ga[:], OP.mult)
                ve.tensor_tensor(ga[:], ga[:], vs[:], OP.add)
                ge.tensor_scalar(ga[:], ga[:], 0.0, 1.0, OP.max, OP.min)
                nc.sync.dma_start(out_d[1][:, sl], ga[:])
                ve.tensor_tensor(ba[:], cc[:], ba[:], OP.mult)
                ve.tensor_tensor(ba[:], ba[:], vs[:], OP.add)
                ge.tensor_scalar(ba[:], ba[:], 0.0, 1.0, OP.max, OP.min)
                nc.sync.dma_start(out_d[2][:, sl], ba[:])

    nc.compile()
    _dedupe_act_table_loads(nc)
    return nc


def _dedupe_act_table_loads(nc):
    """All ACT funcs used here (Ln, Exp, Copy, Relu) live together in the
    'natural_log_exp_and_others' set; collapse the per-function table loads
    the compiler inserted into one load of that set."""
    from concourse.hw_specs import get_activation_tables
    import concourse.mybir as mybir_
    tables = list(get_activation_tables(nc.m.arch).items())
    target = None
    need = {AF.Ln, AF.Exp, AF.Copy, AF.Relu}
    for idx, (name, fns) in enumerate(tables):
        if need.issubset(fns):
            target = idx
            break
    if target is None:
        return
    first = True
    for blk in nc.main_func.blocks:
        keep = []
        for inst in blk.instructions:
            if isinstance(inst, mybir_.InstLoadActFuncSet):
                si = inst.sync_info
                has_sync = si is not None and (len(si.on_wait) > 0 or len(si.on_update) > 0)
                if first or has_sync:
                    inst.act_func_set_id = target
                    first = False
                    keep.append(inst)
                # else: drop duplicate load
            else:
                keep.append(inst)
        if len(keep) != len(blk.instructions):
            blk.instructions[:] = keep


_CACHE = {}


def _get_program(L, R, S):
    key = (L.tobytes(), R.tobytes(), S.tobytes())
    if key not in _CACHE:
        labc, d1 = _curve_coeffs(np.asarray(L, np.float32), 16)
        rgbc, d2 = _curve_coeffs(np.asarray(R, np.float32), 16)
        hsvc, d3 = _curve_coeffs(np.asarray(S, np.float32), 16)
        ssd = np.float32(d1 + np.float32(d2 + d3))
        nc = build_program(labc, rgbc, hsvc)
        _CACHE[key] = (nc, ssd)
    return _CACHE[key]


def _run(nc, img, **spmd_kwargs):
    img = np.ascontiguousarray(np.asarray(img, np.float32))
    shards = [np.ascontiguousarray(img[:, i * HSH:(i + 1) * HSH, :]).reshape(3, P, FREE)
              for i in range(NCORE)]
    in_maps = [{"img": s} for s in shards]
    res = run_bass_kernel_spmd(nc, in_maps, list(range(NCORE)), **spmd_kwargs)
    outs = [res.results[i]["out"].reshape(3, HSH, W) for i in range(NCORE)]
    full = np.concatenate(outs, axis=1)
    return full, res


def kernel(img, L, R, S):
    nc, ssd = _get_program(np.asarray(L, np.float32), np.asarray(R, np.float32),
                           np.asarray(S, np.float32))
    full, _ = _run(nc, img)
    return full, np.float32(ssd)


def kernel_traced(img, L, R, S, **kw):
    """test harness entry: also returns BassKernelResults for timing."""
    nc, ssd = _get_program(np.asarray(L, np.float32), np.asarray(R, np.float32),
                           np.asarray(S, np.float32))
    full, res = _run(nc, img, **kw)
    return (full, np.float32(ssd)), res


# revision 11
# speedup vs baseline: 3.2360x; 3.2360x over previous
"""CURL color-pipeline kernel for 8 TRN2 NeuronCores (Bass/Tile).

Contract: kernel(img, L, R, S) takes the FULL inputs (img (3,2048,2048) f32,
L/R/S curve params), returns (out_image (3,2048,2048) f32, slope_sqr_diff f32
scalar) exactly like the reference.

Strategy:
  - The regulariser ssd and all per-curve scalars (A_c, B_c with
    scale = A + B*x) depend only on the 160 curve parameters -> computed on
    host in float32 with reference-faithful op order, baked into the device
    program as immediates.
  - Image work is purely per-pixel. Shard H across the 8 cores
    (each core: (3, 256, 2048) = 3 planes of [128 partitions x 4096]).
  - Per core, process 4 column-chunks of [128, 1024] through a fused
    DVE/ACT/GPSIMD schedule (no PE: fp32r matmul rounds inputs to 11
    mantissa bits, too coarse).
"""
import numpy as np

import concourse.bacc as bacc
import concourse.bass as bass
import concourse.mybir as mybir
import concourse.tile as tile
from concourse.bass_utils import run_bass_kernel_spmd

F32 = mybir.dt.float32
U8 = mybir.dt.uint8
I32_ = mybir.dt.int32
OP = mybir.AluOpType
AF = mybir.ActivationFunctionType

P = 128          # partitions
FREE = 4096      # free dim per plane per core
CHUNK = 2048     # columns per compute chunk
NCHUNK = FREE // CHUNK
NCORE = 8
H, W = 2048, 2048
HSH = H // NCORE  # rows per core

EPS = np.float32(6.0 / 29.0)
WPS = [np.float32(0.950456), np.float32(1.0), np.float32(1.088754)]
M2 = np.float32([[3.2404542, -0.969266, 0.0556434],
                 [-1.5371385, 1.8760108, -0.2040259],
                 [-0.4985314, 0.041556, 1.0572252]])
GAM_C = np.float32(0.0031308)
GAM_BITS = int(GAM_C.view(np.int32))
I32 = None  # set below

USE_GPSIMD = False


def _curve_coeffs(Pv, n):
    """Per-curve (A, B) with scale = A + B*x, and the ssd term; float32
    semantics faithful to the reference."""
    out = []
    ssd = np.float32(0.0)
    ncurves = Pv.shape[0] // n
    for c in range(ncurves):
        C = np.exp(Pv[c * n:(c + 1) * n].astype(np.float32)).astype(np.float32)
        K = C.shape[0]
        slope = (C[1:] - C[:-1]).astype(np.float32)
        d = np.float32(np.sum(((slope[1:] - slope[:-1]).astype(np.float32) ** 2).astype(np.float32),
                              dtype=np.float32))
        ssd = np.float32(ssd + d)
        s = slope[:-1]
        B = np.float32(np.float32(K - 1) * np.float32(np.sum(s, dtype=np.float32)))
        A = np.float32(C[0] - np.float32(np.dot(s, np.arange(K - 2, dtype=np.float32))))
        out.append((float(A), float(B)))
    return out, ssd


def build_program(labc, rgbc, hsvc):
    """Build the single-core Bass program (same NEFF runs SPMD on all 8).

    SBUF slot plan (all [P, CHUNK] fp32, pool tags, bufs=1 unless noted):
      io: x0 x1 x2 (bufs=2)          raw input channels, die after S1
      Y0 Y1 Y2: y (S1) -> gamma t/p/pw (S2d) -> mx/mn/df (S4) -> ra/v1/s2 ...
      F0 F1 F2: f (S2a) -> rgb (S2c) -> inv/sv/hb (S4) -> u4/u5/cc (S6)
      A0 A1 A2: a_i (S2b) -> gq (S2d) -> hg/hr (S4) -> vs/ga/h1->ba (S5/6)
      B0 B1 B2: b0/xyz (S2b/c) -> r,g,b (S3) -> u1/u2/u3 (S6)
    Exact lifetime order is load-bearing; validated against golden in sim.
    """
    nc = bacc.Bacc("TRN2", target_bir_lowering=False, debug=False)

    img_d = nc.dram_tensor("img", (3, P, FREE), F32, kind="ExternalInput").ap()
    out_d = nc.dram_tensor("out", (3, P, FREE), F32, kind="ExternalOutput").ap()

    with tile.TileContext(nc) as tc:
        import contextlib
        ctx = contextlib.ExitStack()
        with ctx:
            io = ctx.enter_context(tc.tile_pool(name="io", bufs=2))
            wk = ctx.enter_context(tc.tile_pool(name="wk", bufs=1))
            mk = ctx.enter_context(tc.tile_pool(name="mk", bufs=2))
            ac = ctx.enter_context(tc.tile_pool(name="ac", bufs=2))

            ve = nc.vector
            se = nc.scalar
            ge = nc.gpsimd if USE_GPSIMD else nc.vector

            def amr(out, in0, in1, scale, bias):
                a = ac.tile([P, 1], F32, tag="acc", name="accd")
                ve.affine_mul_reduce(out, a[:], in0, in1, scale, bias)

            # Relu bias constants -k for the hsv->rgb windows
            relu_bias = []
            for k in range(1, 6):
                t = ac.tile([P, 1], F32, tag=f"rb{k}", bufs=1, name=f"rb{k}_")
                nc.gpsimd.memset(t[:], float(-k))
                relu_bias.append(t)

            def const_tile(tag, val):
                t = ac.tile([P, 1], F32, tag=tag, bufs=1, name=tag + "_")
                nc.gpsimd.memset(t[:], float(val))
                return t

            eps_b = const_tile("epsb", float(EPS))
            neg_eps_b = const_tile("nepsb", float(-EPS))
            em4_b = const_tile("em4b", 1e-4)
            neg_em4_b = const_tile("nem4b", -1e-4)
            w3l_b = [const_tile(f"w3l{i}",
                                float(np.float32(3.0) * np.float32(np.log(np.float32(WPS[i] ** (1.0 / 3.0))))))
                     for i in range(3)]

            for ci in range(NCHUNK):
                sl = bass.ts(ci, CHUNK)

                def slot(tag, bufs=None, name=None):
                    return wk.tile([P, CHUNK], F32, tag=tag, bufs=bufs,
                                   name=name or (tag + "_"))

                x = [io.tile([P, CHUNK], F32, tag=f"x{c}", name=f"x{c}_") for c in range(3)]
                for c in range(3):
                    nc.sync.dma_start(x[c][:], img_d[c][:, sl])

                # ---- S1: LAB curves  y = clip((x*B+A)*x, 0, 1)   [slots Y]
                y = [slot(f"Y{c}") for c in range(3)]
                for c, (A, B) in enumerate(labc):
                    amr(y[c][:], x[c][:], x[c][:], B, A)
                    ge.tensor_scalar(y[c][:], y[c][:], 0.0, 1.0, OP.max, OP.min)

                # ---- S2a: f's [slots F]; fy=F1 fx=F0 fz=F2
                f = [slot(f"F{i}") for i in range(3)]
                se.activation(f[1][:], y[0][:], AF.Copy, bias=16.0 / 116.0, scale=100.0 / 116.0)
                ve.affine_then_add(f[0][:], y[1][:], f[1][:], 220.0 / 500.0, -110.0 / 500.0)
                ve.affine_then_add(f[2][:], y[2][:], f[1][:], -220.0 / 200.0, 110.0 / 200.0)

                # ---- S2b: a_i in [slots A] (rw->ln->exp in place);
                #           b0->xyz in [slots B]
                av = [slot(f"A{i}") for i in range(3)]
                xyz = [slot(f"B{i}") for i in range(3)]
                for i in range(3):
                    # a_i = (w3*max(f,eps))^3 = exp(3*ln(relu(f-eps)+eps) + 3*ln(w3))
                    se.activation(av[i][:], f[i][:], AF.Relu, bias=neg_eps_b[:], scale=1.0)
                    se.activation(av[i][:], av[i][:], AF.Ln, bias=eps_b[:], scale=1.0)
                    se.activation(av[i][:], av[i][:], AF.Exp, bias=w3l_b[i][:], scale=3.0)
                    # -b0_i = relu(eps - f); sign folded into the mix coefficient
                    se.activation(xyz[i][:], f[i][:], AF.Relu, bias=eps_b[:], scale=-1.0)
                    gamma_i = float(np.float32(3.0) * EPS * EPS * WPS[i])
                    ve.affine_then_add(xyz[i][:], xyz[i][:], av[i][:], -gamma_i, 0.0)

                # ---- S2c: mix -> rgb_j reuses [slots F] (f dead)
                rgb = [slot(f"F{j}", name=f"rgb{j}_") for j in range(3)]
                for j in range(3):
                    se.activation(rgb[j][:], xyz[0][:], AF.Copy, bias=0.0, scale=float(M2[0, j]))
                    ve.affine_then_add(rgb[j][:], xyz[1][:], rgb[j][:], float(M2[1, j]), 0.0)
                    ve.affine_then_add(rgb[j][:], xyz[2][:], rgb[j][:], float(M2[2, j]), 0.0)

                # ---- S2d: gamma; t/p/pw in [slots Y] (y dead), gq=lin in [slots A] (a dead)
                tt = [slot(f"Y{j}", name=f"t{j}_") for j in range(3)]
                gq = [slot(f"A{j}", name=f"gq{j}_") for j in range(3)]
                for j in range(3):
                    se.activation(tt[j][:], rgb[j][:], AF.Relu, bias=neg_em4_b[:], scale=1.0)
                    se.activation(tt[j][:], tt[j][:], AF.Ln, bias=em4_b[:], scale=1.0)
                    se.activation(tt[j][:], tt[j][:], AF.Exp, scale=1.0 / 2.4)
                    se.activation(tt[j][:], tt[j][:], AF.Copy, bias=-0.055, scale=1.055)
                    se.activation(gq[j][:], rgb[j][:], AF.Copy, bias=0.0, scale=12.92)
                    m = mk.tile([P, CHUNK], U8, tag="gm", name="gm_")
                    ve.tensor_scalar(m[:], rgb[j][:], float(GAM_C), None, OP.is_gt)
                    ve.copy_predicated(gq[j][:], m[:], tt[j][:])

                # ---- S3: RGB curves -> r,g,b in [slots B] (xyz dead)
                (A0, B0), (A1, B1), (A2, B2) = rgbc
                rcv = [slot(f"B{j}", name=f"rc{j}_") for j in range(3)]
                amr(rcv[0][:], gq[0][:], gq[0][:], B0, A0)
                ge.tensor_scalar(gq[1][:], gq[1][:], 0.0, 1.0, OP.max, OP.min)
                amr(rcv[1][:], gq[1][:], gq[1][:], B1, A1)
                ge.tensor_scalar(gq[2][:], gq[2][:], 0.0, 1.0, OP.max, OP.min)
                amr(rcv[2][:], gq[2][:], gq[2][:], B2, A2)
                for j in range(3):
                    ve.tensor_scalar(rcv[j][:], rcv[j][:], 1e-9, 1.0, OP.max, OP.min)
                r, g, b = rcv

                # ---- S4: rgb -> hsv; mx/mn/df in [slots Y] (t dead);
                #          inv/sv/hb in [slots F] (rgb dead); hg/hr in A1/A2 (gq dead
                #          after S3; gq0 dead after its amr)
                mx = slot("Y0", name="mx_")
                mn = slot("Y1", name="mn_")
                ve.tensor_tensor(mx[:], r[:], g[:], OP.max)
                ve.tensor_tensor(mx[:], mx[:], b[:], OP.max)
                ve.tensor_tensor(mn[:], r[:], g[:], OP.min)
                ve.tensor_tensor(mn[:], mn[:], b[:], OP.min)
                df = slot("Y2", name="df_")
                ge.tensor_tensor(df[:], mx[:], mn[:], OP.subtract)
                ge.tensor_scalar(df[:], df[:], 1e-10, None, OP.add)
                inv = slot("F0", name="inv_")
                ve.reciprocal_approx_fast(inv[:], df[:])
                sv = slot("F1", name="sv_")
                ve.reciprocal_approx_fast(sv[:], mx[:])
                ve.tensor_tensor(sv[:], df[:], sv[:], OP.mult)   # s = df/mx

                hb = slot("F2", name="hb_")
                ge.tensor_tensor(hb[:], r[:], g[:], OP.subtract)
                ve.scalar_tensor_tensor(hb[:], hb[:], 1.0 / 6.0, inv[:], OP.mult, OP.mult)
                se.activation(hb[:], hb[:], AF.Copy, bias=2.0 / 3.0, scale=1.0)
                hg = slot("A0", name="hg_")
                ge.tensor_tensor(hg[:], b[:], r[:], OP.subtract)
                ve.scalar_tensor_tensor(hg[:], hg[:], 1.0 / 6.0, inv[:], OP.mult, OP.mult)
                se.activation(hg[:], hg[:], AF.Copy, bias=1.0 / 3.0, scale=1.0)
                hr = slot("A1", name="hr_")
                ge.tensor_tensor(hr[:], g[:], b[:], OP.subtract)
                ve.scalar_tensor_tensor(hr[:], hr[:], 1.0 / 6.0, inv[:], OP.mult, OP.mult)
                ve.scalar_tensor_tensor(hr[:], hr[:], 0.0, hr[:], OP.is_lt, OP.add)
                mg = mk.tile([P, CHUNK], U8, tag="mg", name="mg_")
                ve.tensor_tensor(mg[:], g[:], mx[:], OP.is_equal)
                mr = mk.tile([P, CHUNK], U8, tag="mr", name="mr_")
                ve.tensor_tensor(mr[:], r[:], mx[:], OP.is_equal)
                ve.copy_predicated(hb[:], mg[:], hg[:])
                ve.copy_predicated(hb[:], mr[:], hr[:])
                # h = hb(F2) ; v = mx(Y0) ; s = sv(F1)

                # ---- S5: HSV curves; h1 in A2 (gq2 dead); s1/s2 in Y2 (df dead);
                #          v1 in Y1 (mn dead)
                (Ah, Bh), (As1, Bs1), (As2, Bs2), (Av, Bv) = hsvc
                h1 = slot("A2", name="h1_")
                amr(h1[:], hb[:], hb[:], Bh, Ah)
                ge.tensor_scalar(h1[:], h1[:], 0.0, 1.0, OP.max, OP.min)
                s2t = slot("Y2", name="s2_")
                amr(s2t[:], h1[:], sv[:], Bs1, As1)
                ve.tensor_scalar(s2t[:], s2t[:], 0.0, 1.0, OP.max, OP.min)
                amr(s2t[:], s2t[:], s2t[:], Bs2, As2)
                ve.tensor_scalar(s2t[:], s2t[:], 0.0, 1.0, OP.max, OP.min)
                v1 = slot("Y1", name="v1_")
                amr(v1[:], mx[:], mx[:], Bv, Av)
                ge.tensor_scalar(v1[:], v1[:], 0.0, 1.0, OP.max, OP.min)

                # ---- S6: hsv -> rgb
                # u1..u3 in [slots B] (r,g,b dead), u4 in F0 (inv dead), u5 in F1 (sv dead)
                u1 = slot("B0", name="u1_"); u2 = slot("B1", name="u2_")
                u3 = slot("B2", name="u3_"); u4 = slot("F0", name="u4_")
                u5 = slot("F1", name="u5_")
                for k, ut in enumerate((u1, u2, u3, u4, u5), start=1):
                    se.activation(ut[:], h1[:], AF.Relu, bias=relu_bias[k - 1][:], scale=6.0)
                cc = slot("F2", name="cc_")      # hb dead (bufs=2: crosses to DMA epoch)
                ve.tensor_tensor(cc[:], v1[:], s2t[:], OP.mult)
                vs = slot("A0", name="vs_")              # hg dead
                ve.affine_then_add(vs[:], cc[:], v1[:], -1.0, 0.0)

                ga = slot("A1", name="ga_")      # hr dead
                ve.scalar_tensor_tensor(ga[:], h1[:], 6.0, u1[:], OP.mult, OP.subtract)
                ve.affine_then_add(ga[:], u3[:], ga[:], -1.0, 0.0)
                ve.affine_then_add(ga[:], u4[:], ga[:], 1.0, 0.0)
                ra = slot("Y0", name="ra_")      # mx dead (after v1 amr)
                ge.tensor_tensor(ra[:], u2[:], u1[:], OP.subtract)
                ve.affine_then_add(ra[:], u4[:], ra[:], 1.0, 0.0)
                ve.affine_then_add(ra[:], u5[:], ra[:], -1.0, 0.0)
                ba = slot("A2", name="ba_")      # h1 dead (after u_k + ga STT)
                ge.tensor_tensor(ba[:], u2[:], u3[:], OP.subtract)
                ve.affine_then_add(ba[:], u5[:], ba[:], -1.0, 0.0)

                ve.tensor_tensor(ra[:], cc[:], ra[:], OP.mult)
                ve.tensor_tensor(ra[:], ra[:], v1[:], OP.add)
                ge.tensor_scalar(ra[:], ra[:], 0.0, 1.0, OP.max, OP.min)
                nc.sync.dma_start(out_d[0][:, sl], ra[:])
                ve.tensor_tensor(ga[:], cc[:], ga[:], OP.mult)
                ve.tensor_tensor(ga[:], ga[:], vs[:], OP.add)
                ge.tensor_scalar(ga[:], ga[:], 0.0, 1.0, OP.max, OP.min)
                nc.sync.dma_start(out_d[1][:, sl], ga[:])
                ve.tensor_tensor(ba[:], cc[:], ba[:], OP.mult)
                ve.tensor_tensor(ba[:], ba[:], vs[:], OP.add)
                ge.tensor_scalar(ba[:], ba[:], 0.0, 1.0, OP.max, OP.min)
                nc.sync.dma_start(out_d[2][:, sl], ba[:])

    nc.compile()
    _dedupe_act_table_loads(nc)
    return nc


def _dedupe_act_table_loads(nc):
    """All ACT funcs used here (Ln, Exp, Copy, Relu) live together in the
    'natural_log_exp_and_others' set; collapse the per-function table loads
    the compiler inserted into one load of that set."""
    from concourse.hw_specs import get_activation_tables
    import concourse.mybir as mybir_
    tables = list(get_activation_tables(nc.m.arch).items())
    target = None
    need = {AF.Ln, AF.Exp, AF.Copy, AF.Relu}
    for idx, (name, fns) in enumerate(tables):
        if need.issubset(fns):
            target = idx
            break
    if target is None:
        return
    first = True
    for blk in nc.main_func.blocks:
        keep = []
        for inst in blk.instructions:
            if isinstance(inst, mybir_.InstLoadActFuncSet):
                si = inst.sync_info
                has_sync = si is not None and (len(si.on_wait) > 0 or len(si.on_update) > 0)
                if first or has_sync:
                    inst.act_func_set_id = target
                    first = False
                    keep.append(inst)
                # else: drop duplicate load
            else:
                keep.append(inst)
        if len(keep) != len(blk.instructions):
            blk.instructions[:] = keep


_CACHE = {}


def _get_program(L, R, S):
    key = (L.tobytes(), R.tobytes(), S.tobytes())
    if key not in _CACHE:
        labc, d1 = _curve_coeffs(np.asarray(L, np.float32), 16)
        rgbc, d2 = _curve_coeffs(np.asarray(R, np.float32), 16)
        hsvc, d3 = _curve_coeffs(np.asarray(S, np.float32), 16)
        ssd = np.float32(d1 + np.float32(d2 + d3))
        nc = build_program(labc, rgbc, hsvc)
        _CACHE[key] = (nc, ssd)
    return _CACHE[key]


def _run(nc, img, **spmd_kwargs):
    img = np.ascontiguousarray(np.asarray(img, np.float32))
    shards = [np.ascontiguousarray(img[:, i * HSH:(i + 1) * HSH, :]).reshape(3, P, FREE)
              for i in range(NCORE)]
    in_maps = [{"img": s} for s in shards]
    res = run_bass_kernel_spmd(nc, in_maps, list(range(NCORE)), **spmd_kwargs)
    outs = [res.results[i]["out"].reshape(3, HSH, W) for i in range(NCORE)]
    full = np.concatenate(outs, axis=1)
    return full, res


def kernel(img, L, R, S):
    nc, ssd = _get_program(np.asarray(L, np.float32), np.asarray(R, np.float32),
                           np.asarray(S, np.float32))
    full, _ = _run(nc, img)
    return full, np.float32(ssd)


def kernel_traced(img, L, R, S, **kw):
    """test harness entry: also returns BassKernelResults for timing."""
    nc, ssd = _get_program(np.asarray(L, np.float32), np.asarray(R, np.float32),
                           np.asarray(S, np.float32))
    full, res = _run(nc, img, **kw)
    return (full, np.float32(ssd)), res
